# revision 1
# baseline (speedup 1.0000x reference)
"""Nearest-neighbor VQ tokenizer on 8 Trainium2 NeuronCores.

Sharding: codebook-parallel. Each core holds ALL 4096 tokens and a
2048-code shard of the [16384, 256] codebook. On-device, each core
computes s = 2*x@c^T - |c|^2 (argmax_n s == argmin_n dist) and finds
per-token top-1 value+index with the DVE max/max_index ops reading
PSUM directly. The host reduces the 8 per-core candidate pairs.

Precision: dot products run on the PE as fp16 hi/lo split matmuls
(xh*ch + xh*cl + xl*ch into fp32 PSUM), carrying ~2^-22 relative
error -- verified to reproduce the fp32 reference argmin exactly --
at 1/4 the PE cost of native fp32 matmul. The -|c|^2 row enters the
same PSUM accumulation as a K=2 matmul of fp16 hi/lo rows against an
all-ones stationary vector.

Pipelining: fp16 operands are built in natural layout (ScalarE casts,
VectorE residuals) and transposed to [d, token]/[d, code] by DMA
xbar transposes, which are descriptor-bound -- so the codebook side is
split into 4 chunk tiles and the token side into 8 groups, letting
matmuls start as soon as the first chunks land. The c2-row assembly
DMAs ride the ScalarE HWDGE rings to dodge head-of-line blocking
behind the transposes on the sync rings.

Math per token t, code n:
    dist[t,n] = |x_t|^2 + |c_n|^2 - 2 x_t.c_n = x2[t] - s[t,n]
    mind[t]   = x2[t] - max_n s[t,n];  idx[t] = argmax_n s[t,n]
"""
import sys
import types
from contextlib import ExitStack

import numpy as np

# If the host env sets BASS_TRACE but this image lacks antenv.axon_hooks,
# run_bass_kernel_spmd would die on the import. Pre-register a no-op hook
# module so tracing degrades gracefully instead.
try:
    import antenv.axon_hooks  # noqa: F401
except ImportError:
    _hooks = types.ModuleType("antenv.axon_hooks")
    _hooks._h = [None]
    _hooks.set_axon_ntff_profile_hook = lambda h: _hooks._h.__setitem__(0, h)
    _hooks.get_axon_ntff_profile_hook = lambda: _hooks._h[0]
    sys.modules["antenv.axon_hooks"] = _hooks

import concourse.bass as bass
import concourse.bacc as bacc
import concourse.tile as tile
from concourse import masks, mybir
from concourse.tile_rust import add_dep_helper
from concourse.bass_utils import run_bass_kernel_spmd

F32 = mybir.dt.float32
F16 = mybir.dt.float16
U32 = mybir.dt.uint32
AF = mybir.ActivationFunctionType

B, S, D = 4, 1024, 256
NTOK = B * S              # 4096
NCODES = 16384
NCORES = 8
NSHARD = NCODES // NCORES  # 2048 codes per core
P = 128
MT = NTOK // P            # 32 token tiles
IT = NSHARD // P          # 16 code tiles
KT = D // P               # 2 contraction tiles
NJ = NSHARD // 512        # 4 psum 512-chunks
NG = 8                    # x-side processing groups
GM = MT // NG             # token tiles per group
DIST_THRESHOLD = 512.0
NO_CODE_ID = -1

_CACHE = {}
LAST_RESULTS = None


def _build():
    nc = bacc.Bacc(
        "TRN2", target_bir_lowering=False, debug=False, enable_asserts=False
    )
    x_d = nc.dram_tensor("x", [NTOK, D], F32, kind="ExternalInput").ap()
    c_d = nc.dram_tensor("codes", [NSHARD, D], F32, kind="ExternalInput").ap()
    mind_d = nc.dram_tensor("mind", [P, MT], F32, kind="ExternalOutput").ap()
    idx_d = nc.dram_tensor("idx", [P, MT], U32, kind="ExternalOutput").ap()

    with tile.TileContext(nc) as tc, ExitStack() as ctx:
        sb = ctx.enter_context(tc.tile_pool(name="sb", bufs=1))
        sq_pool = ctx.enter_context(tc.tile_pool(name="sq", bufs=2))

        cn = sb.tile([P, IT, D], F32)       # cn[p, i, d] = codes[p*IT+i, d]
        cnh = sb.tile([P, IT, D], F16)      # fp16(2*codes)
        cnl = sb.tile([P, IT, D], F16)      # 2*codes - cnh
        # transposed codes, split front/back so matmuls can start after
        # only the front half has landed: [dl, i*2+k, q] per half
        cTh_h = [sb.tile([P, IT * KT // 2, P], F16, name=f"cTh{h}") for h in range(2)]
        cTl_h = [sb.tile([P, IT * KT // 2, P], F16, name=f"cTl{h}") for h in range(2)]
        xn_g = [sb.tile([P, GM, D], F32, name=f"xn{g}") for g in range(NG)]
        xnh_g = [sb.tile([P, GM, D], F16, name=f"xnh{g}") for g in range(NG)]
        xnl_g = [sb.tile([P, GM, D], F16, name=f"xnl{g}") for g in range(NG)]
        xTh_g = [
            sb.tile([P, GM * KT, P], F16, name=f"xTh{g}") for g in range(NG)
        ]
        xTl_g = [
            sb.tile([P, GM * KT, P], F16, name=f"xTl{g}") for g in range(NG)
        ]
        c2row = sb.tile([1, NSHARD], F32)   # -|c_n|^2
        c2row2 = sb.tile([2, NSHARD], F16)  # hi/lo rows of -|c_n|^2
        c2h_tmp = sb.tile([1, NSHARD], F16)
        c2l_tmp = sb.tile([1, NSHARD], F16)
        ones2 = sb.tile([2, P], F16)
        ident = sb.tile([P, P], F32)
        x2all = sb.tile([P, MT], F32)       # |x_t|^2
        c2all = sb.tile([P, IT], F32)
        c2T = sb.tile([IT, P], F32)
        val8 = sb.tile([P, MT * 8], F32)
        idx8 = sb.tile([P, MT * 8], U32)
        mind_sb = sb.tile([P, MT], F32)
        idx_sb = sb.tile([P, MT], U32)

        # Big clean loads first (p-outer layout: one contiguous descriptor
        # per partition), ahead of everything in the sync DMA rings.
        nc.scalar.dma_start(cn[:], c_d.rearrange("(p i) d -> p i d", i=IT))
        for g in range(2):
            nc.sync.dma_start(
                xn_g[g][:],
                x_d.rearrange("(p m) d -> p m d", m=MT)[
                    :, g * GM : (g + 1) * GM, :
                ],
            )
        nc.gpsimd.memset(ones2[:], 1.0)
        masks.make_identity(nc, ident[:])

        # ---- codes side ----
        # cnh = fp16(2c) (exact x2 scale), cnl = 2c - cnh, c2 = sum c^2
        HI = IT // 2

        def codes_chain(h):
            hs = slice(h * HI, (h + 1) * HI)
            nc.scalar.activation(cnh[:, hs, :], cn[:, hs, :], AF.Copy, scale=2.0)
            nc.vector.scalar_tensor_tensor(
                out=cnl[:, hs, :], in0=cn[:, hs, :], scalar=2.0,
                in1=cnh[:, hs, :],
                op0=mybir.AluOpType.mult, op1=mybir.AluOpType.subtract,
            )
            nc.sync.dma_start_transpose(cTh_h[h][:], cnh[:, hs, :])
            nc.sync.dma_start_transpose(cTl_h[h][:], cnl[:, hs, :])

        def c2_chain():
            for i in range(IT):
                sq = sq_pool.tile([P, D], F32, tag="sq", name="sq")
                nc.scalar.activation(
                    sq[:], cn[:, i, :], AF.Square,
                    accum_out=c2all[:, i : i + 1],
                )
            c2_body()

        # ---- c2 row: transpose [P, IT] -> [IT, P] on the PE, negate, and
        # assemble the [1, NSHARD] row + fp16 hi/lo rows. The tiny DMAs go
        # through the ScalarE HWDGE rings (empty) to avoid head-of-line
        # blocking behind the transposes in the sync rings.
        c2_refs = {}

        def c2_body():
            with ExitStack() as sctx:
                tp = sctx.enter_context(
                    tc.tile_pool(name="tp", bufs=1, space="PSUM")
                )
                pc2 = tp.tile([IT, P], F32, tag="tp")
                nc.tensor.matmul(
                    pc2[:], c2all[:], ident[:], is_transpose=True
                )
                nc.scalar.mul(c2T[:], pc2[:], -1.0)
            nc.scalar.dma_start(
                c2row[0:1, :].rearrange("a (i q) -> a i q", q=P), c2T[:]
            )
            nc.vector.tensor_copy(c2h_tmp[0:1, :], c2row[0:1, :])
            c2_refs["l"] = nc.vector.tensor_sub(
                c2l_tmp[0:1, :], c2row[0:1, :], c2h_tmp[0:1, :]
            )
            nc.scalar.dma_start(c2row2[0:1, :], c2h_tmp[0:1, :])
            c2_refs["d"] = nc.scalar.dma_start(c2row2[1:2, :], c2l_tmp[0:1, :])

        def x_chain(g):
            act_i = nc.scalar.activation(xnh_g[g][:], xn_g[g][:], AF.Copy)
            sub_i = nc.vector.tensor_sub(
                xnl_g[g][:], xn_g[g][:], xnh_g[g][:]
            )
            if g == 1:
                # Pin the c2-row assembly ahead of later x-side work in the
                # ScalarE/VectorE streams: the scheduler otherwise floats
                # it behind, starving the first PSUM groups.
                add_dep_helper(
                    act_i.ins, c2_refs["d"].ins, sync=False,
                    reason="c2 rows before x prep on ScalarE",
                )
                add_dep_helper(
                    sub_i.ins, c2_refs["l"].ins, sync=False,
                    reason="c2 rows before x prep on VectorE",
                )
            nc.sync.dma_start_transpose(xTh_g[g][:], xnh_g[g][:])
            nc.sync.dma_start_transpose(xTl_g[g][:], xnl_g[g][:])
            for lm in range(GM):
                m = g * GM + lm
                sq = sq_pool.tile([P, D], F32, tag="sq", name="sq")
                nc.scalar.activation(
                    sq[:], xn_g[g][:, lm, :], AF.Square,
                    accum_out=x2all[:, m : m + 1],
                )

        codes_chain(0)
        x_chain(0)
        codes_chain(1)
        c2_chain()
        x_chain(1)
        for g in range(2, NG):
            nc.sync.dma_start(
                xn_g[g][:],
                x_d.rearrange("(p m) d -> p m d", m=MT)[
                    :, g * GM : (g + 1) * GM, :
                ],
            )

        with ExitStack() as sctx:
            sp = sctx.enter_context(
                tc.tile_pool(name="sp", bufs=2, space="PSUM")
            )
            for g in range(NG):
                if g + 2 < NG:
                    x_chain(g + 2)
                for lm in range(GM):
                    m = g * GM + lm
                    s = sp.tile([P, NSHARD], F32, tag="s", name="s")
                    cThv = [
                        t[:].rearrange("p (i k) q -> p k i q", k=KT)
                        for t in cTh_h
                    ]
                    cTlv = [
                        t[:].rearrange("p (i k) q -> p k i q", k=KT)
                        for t in cTl_h
                    ]
                    terms = [
                        (xTh_g[g][:, lm * KT + 0, :], cThv, 0),
                        (xTh_g[g][:, lm * KT + 1, :], cThv, 1),
                        (xTh_g[g][:, lm * KT + 0, :], cTlv, 0),
                        (xTh_g[g][:, lm * KT + 1, :], cTlv, 1),
                        (xTl_g[g][:, lm * KT + 0, :], cThv, 0),
                        (xTl_g[g][:, lm * KT + 1, :], cThv, 1),
                    ]
                    for ti, (lhsT, rhsv, k) in enumerate(terms):
                        for j in range(NJ):
                            jj = j % 2
                            nc.tensor.matmul(
                                s[:, j * 512 : (j + 1) * 512],
                                lhsT,
                                rhsv[j // 2][:, k, 4 * jj : 4 * jj + 4, :],
                                start=(ti == 0), stop=False,
                            )
                    for j in range(NJ):
                        nc.tensor.matmul(
                            s[:, j * 512 : (j + 1) * 512],
                            ones2[0:2, :],
                            c2row2[0:2, j * 512 : (j + 1) * 512],
                            start=False, stop=True,
                        )
                    nc.vector.max(val8[:, m * 8 : m * 8 + 8], s[:])
                    nc.vector.max_index(
                        idx8[:, m * 8 : m * 8 + 8],
                        val8[:, m * 8 : m * 8 + 8], s[:],
                    )

        # Top-1 extraction: mind = x2 - max_s, idx = argmax position.
        v0 = val8[:].rearrange("p (m e) -> p m e", e=8)[:, :, 0]
        i0 = idx8[:].rearrange("p (m e) -> p m e", e=8)[:, :, 0]
        nc.vector.tensor_sub(mind_sb[:], x2all[:], v0)
        nc.vector.tensor_copy(idx_sb[:], i0)
        nc.sync.dma_start(mind_d[:], mind_sb[:])
        nc.sync.dma_start(idx_d[:], idx_sb[:])

    nc.compile()
    return nc


def kernel(x, codes, is_active=None, **_):
    global LAST_RESULTS
    if "nc" not in _CACHE:
        _CACHE["nc"] = _build()
    nc = _CACHE["nc"]

    x_flat = np.ascontiguousarray(
        np.asarray(x, dtype=np.float32).reshape(NTOK, D)
    )
    codes_np = np.asarray(codes, dtype=np.float32)
    in_maps = [
        {
            "x": x_flat,
            "codes": np.ascontiguousarray(
                codes_np[c * NSHARD : (c + 1) * NSHARD]
            ),
        }
        for c in range(NCORES)
    ]
    try:
        LAST_RESULTS = run_bass_kernel_spmd(nc, in_maps, list(range(NCORES)))
    except Exception:
        # One retry: the axon-tunneled device occasionally reports a
        # transient NRT_EXEC_UNIT_UNRECOVERABLE on the first dispatch.
        LAST_RESULTS = run_bass_kernel_spmd(nc, in_maps, list(range(NCORES)))
    res = LAST_RESULTS.results

    # Host-side reduce over the 8 codebook shards.
    # Token layout: [p, m] -> token p*MT+m (p-outer contiguous loads).
    # Code positions n in the transposed layout map to id (n%128)*IT+n//128.
    code_perm = (np.arange(NSHARD) % P) * IT + np.arange(NSHARD) // P
    minds = np.stack([r["mind"].reshape(NTOK) for r in res])
    idxs = np.stack(
        [
            code_perm[r["idx"].reshape(NTOK).astype(np.int64)] + c * NSHARD
            for c, r in enumerate(res)
        ]
    )
    best = np.argmin(minds, axis=0)
    ar = np.arange(NTOK)
    mind = minds[best, ar]
    idx = idxs[best, ar]
    ok = mind <= DIST_THRESHOLD
    idxs_out = np.where(ok, idx, NO_CODE_ID).astype(np.int32).reshape(B, S)
    mind_out = mind.astype(np.float32).reshape(B, S)
    return idxs_out, mind_out



# revision 3
# speedup vs baseline: 1.0195x; 1.0195x over previous
"""Nearest-neighbor VQ tokenizer on 8 Trainium2 NeuronCores.

Device: one fp8(e4m3) DoubleRow matmul pass computes the dot scores
d[t, n] = 2*x_t.c_n for each core's 2048-code shard (K=256 contracted
per instruction via the [128, 2, *] k-tile packing; 2x fp16 PE
throughput). No |c|^2 on the PE - the c2-row accumulation would
double PE time (column-streaming-bound, not MAC-bound).

Reduction: ISA limits shape it - only DVE can max, at most one
tensor-op input may read PSUM, Pool/DMA cannot touch PSUM at all.
Pipelined at half-tile granularity over 4 x [128, 1024] PSUM buffers,
per 128-token tile g:
    mm B0, mm B1                 (cols 1024:2047 -> psumB)
    ACT: evac psumB -> ub fp16   (frees psumB; overlaps A matmuls)
    mm A0, mm A1                 (cols 0:1023 -> psumA)
    DVE: w[g] = max(psumA, ub)   (one PSUM operand; frees psumA)
so the three engines run ~95% duty. w ships as fp8 (halves the
output-DMA tail); slot j = max(d[:, j], d[:, j+1024]).

Codebook order (host-side): each shard is sorted by |c|^2 and the
sorted pair 2j, 2j+1 is placed at columns j, j+1024, so a fold slot
covers two codes of nearly equal |c|^2. The host turns slot dot-
maxima into score bounds (EPS covers fp8 screening error, measured
|err| <= 7.6, plus fp8/fp16 rounding of w):
    UB(j) = w[j] + err8 + EPS - c2min(j)
    LB(j) = w[j] - err8 - EPS - c2max(j)
keeps slots with UB >= max LB (~40 codes/token), and re-scores them
exactly in fp32 - reproducing the reference argmin and min distance
bit-for-bit on the idx path.

Inputs are pre-quantized/transposed/sharded on the host; the device
runs no casts and no transposes:
  xt [128, 2, 4096] fp8   xt[p, k, t] = fp8(2*x[t, 128k+p])
  ct [128, 2, 2048] fp8   ct[p, k, n] = fp8(codes[perm[n], 128k+p])
"""
import sys
import types
from contextlib import ExitStack

import ml_dtypes
import numpy as np

try:
    import antenv.axon_hooks  # noqa: F401
except ImportError:
    _hooks = types.ModuleType("antenv.axon_hooks")
    _hooks._h = [None]
    _hooks.set_axon_ntff_profile_hook = lambda h: _hooks._h.__setitem__(0, h)
    _hooks.get_axon_ntff_profile_hook = lambda: _hooks._h[0]
    sys.modules["antenv.axon_hooks"] = _hooks

import concourse.bacc as bacc
import concourse.tile as tile
from concourse import mybir
from concourse.bass_utils import run_bass_kernel_spmd

AF = mybir.ActivationFunctionType
F32 = mybir.dt.float32
F16 = mybir.dt.float16
F8 = mybir.dt.float8e4
DR = mybir.MatmulPerfMode.DoubleRow
MAX = mybir.AluOpType.max
FP8NP = ml_dtypes.float8_e4m3

B, S, D = 4, 1024, 256
NTOK = B * S               # 4096
NCODES = 16384
NCORES = 8
NSHARD = NCODES // NCORES  # 2048 codes per core
P = 128
KT = D // P                # 2 k-tiles = one DoubleRow pair (K=256)
MT = NTOK // P             # 32 token tiles; token t = g*128 + q
W = 1024                   # slots; slot j = sorted codes {2j, 2j+1}
DIST_THRESHOLD = 512.0
NO_CODE_ID = -1
EPS = 12.0                 # score-error bound (measured max 7.6 + fp16)

_CACHE = {}
LAST_RESULTS = None


def _build():
    nc = bacc.Bacc(
        "TRN2", target_bir_lowering=False, debug=False, enable_asserts=False
    )
    xt_d = nc.dram_tensor("xt", [P, KT, NTOK], F8, kind="ExternalInput").ap()
    ct_d = nc.dram_tensor("ct", [P, KT, NSHARD], F8, kind="ExternalInput").ap()
    w_d = nc.dram_tensor("w", [P, MT, W], F8, kind="ExternalOutput").ap()

    with tile.TileContext(nc) as tc, ExitStack() as ctx:
        sb = ctx.enter_context(tc.tile_pool(name="sb", bufs=1))
        fold = ctx.enter_context(tc.tile_pool(name="fold", bufs=3))

        xt = sb.tile([P, KT, NTOK], F8)
        ct = sb.tile([P, KT, NSHARD], F8)
        w_acc = sb.tile([P, MT, W], F8)

        # Critical-path loads first: tile 0 needs ct's B half and the
        # first token tiles, then ct's A half; the rest follows.
        nc.scalar.dma_start(ct[:, :, W : 2 * W], ct_d[:, :, W : 2 * W])
        nc.sync.dma_start(xt[:, :, 0:512], xt_d[:, :, 0:512])
        nc.scalar.dma_start(ct[:, :, 0:W], ct_d[:, :, 0:W])
        for lo, hi in ((512, 1536), (1536, 2816), (2816, NTOK)):
            nc.sync.dma_start(xt[:, :, lo:hi], xt_d[:, :, lo:hi])

        with ExitStack() as sctx:
            sp = sctx.enter_context(tc.tile_pool(name="sp", bufs=2, space="PSUM"))
            for g in range(MT):
                lhsT = xt[:, :, g * P : (g + 1) * P]
                pb = sp.tile([P, W], F32, tag="pb", name="pb")
                for j in (0, 1):
                    nc.tensor.matmul(
                        pb[:, j * 512 : (j + 1) * 512],
                        lhsT,
                        ct[:, :, W + j * 512 : W + (j + 1) * 512],
                        start=True, stop=True, perf_mode=DR,
                    )
                ub = fold.tile([P, W], F16, tag="ub", name="ub")
                nc.scalar.activation(ub[:], pb[:], AF.Copy)
                pa = sp.tile([P, W], F32, tag="pa", name="pa")
                for j in (0, 1):
                    nc.tensor.matmul(
                        pa[:, j * 512 : (j + 1) * 512],
                        lhsT,
                        ct[:, :, j * 512 : (j + 1) * 512],
                        start=True, stop=True, perf_mode=DR,
                    )
                nc.vector.tensor_tensor(w_acc[:, g, :], pa[:], ub[:], op=MAX)
                if g % 2 == 1:
                    eng = nc.sync if (g // 2) % 2 == 0 else nc.scalar
                    eng.dma_start(
                        w_d[:, g - 1 : g + 1, :], w_acc[:, g - 1 : g + 1, :]
                    )
    nc.compile()
    return nc


def _prep_inputs(x_flat, codes_np):
    xq = (2.0 * x_flat).astype(FP8NP)
    xt8 = np.ascontiguousarray(xq.T.reshape(KT, P, NTOK).transpose(1, 0, 2))
    in_maps = []
    perms = []
    for c in range(NCORES):
        shard = codes_np[c * NSHARD : (c + 1) * NSHARD]
        c2 = np.sum(shard.astype(np.float32) ** 2, axis=1)
        order = np.argsort(c2, kind="stable")
        # column n holds sorted code 2*(n % W) + n // W, so fold slot j
        # (cols j and j+W) covers the c2-adjacent pair order[2j], order[2j+1]
        n = np.arange(NSHARD)
        perm = order[2 * (n % W) + n // W]
        cq = shard[perm].astype(FP8NP)
        ct8 = np.ascontiguousarray(cq.T.reshape(KT, P, NSHARD).transpose(1, 0, 2))
        in_maps.append({"xt": xt8, "ct": ct8})
        perms.append(perm)
    return in_maps, perms


def kernel(x, codes, is_active=None, **_):
    global LAST_RESULTS
    if "nc" not in _CACHE:
        _CACHE["nc"] = _build()
    nc = _CACHE["nc"]

    x_flat = np.ascontiguousarray(np.asarray(x, dtype=np.float32).reshape(NTOK, D))
    codes_np = np.asarray(codes, dtype=np.float32)
    in_maps, perms = _prep_inputs(x_flat, codes_np)
    try:
        LAST_RESULTS = run_bass_kernel_spmd(nc, in_maps, list(range(NCORES)))
    except Exception:
        LAST_RESULTS = run_bass_kernel_spmd(nc, in_maps, list(range(NCORES)))
    res = LAST_RESULTS.results

    # w[c][q, g, j] = max(2x.c over slot j's two codes) for token g*128+q
    Wd = np.stack([np.asarray(r["w"], np.float32) for r in res])  # [8, P, MT, W]
    Wt = Wd.transpose(0, 2, 1, 3).reshape(NCORES, NTOK, W)

    c2all = np.sum(codes_np * codes_np, axis=1)
    slot_codes = np.stack(
        [np.stack([p[:W], p[W:]], axis=1) + c * NSHARD for c, p in enumerate(perms)]
    )  # [8, W, 2] global code ids
    slot_c2 = c2all[slot_codes]                  # [8, W, 2]
    c2_min = slot_c2.min(axis=2)
    c2_max = slot_c2.max(axis=2)

    # w is fp8: add its elementwise half-ulp-ish rounding bound on top of
    # the fp8-matmul screening bound EPS.
    err8 = np.abs(Wt) * (2.0 ** -4)
    ub = Wt + err8 + (EPS - c2_min)[:, None, :]  # [8, NTOK, W]
    lb = Wt - err8 - (EPS + c2_max)[:, None, :]
    best_lb = lb.max(axis=(0, 2))                # [NTOK]
    mask = ub >= best_lb[None, :, None]
    cc, tt, jj = np.nonzero(mask)
    cand = slot_codes[cc, jj].reshape(-1)
    tok = np.repeat(tt, 2)

    x2 = np.sum(x_flat * x_flat, axis=1)
    dist = np.empty(len(cand), np.float32)
    CH = 1 << 20
    for lo_i in range(0, len(cand), CH):
        sl = slice(lo_i, min(lo_i + CH, len(cand)))
        dots = np.einsum(
            "nd,nd->n", x_flat[tok[sl]], codes_np[cand[sl]], dtype=np.float32
        )
        dist[sl] = x2[tok[sl]] + c2all[cand[sl]] - 2.0 * dots

    mind = np.full(NTOK, np.inf, np.float32)
    np.minimum.at(mind, tok, dist)
    is_min = dist == mind[tok]
    idx = np.full(NTOK, NCODES, np.int64)
    np.minimum.at(idx, tok[is_min], cand[is_min])

    ok = mind <= DIST_THRESHOLD
    idxs_out = np.where(ok, idx, NO_CODE_ID).astype(np.int32).reshape(B, S)
    mind_out = mind.astype(np.float32).reshape(B, S)
    return idxs_out, mind_out
